# revision 82
# baseline (speedup 1.0000x reference)
"""BlanchotianAttention TRN2 kernel: 8 NeuronCores, data-parallel over batch (2)
x tensor-parallel over heads (4 heads/core).

Design (pair-phased schedule, cost-model driven):
  - Host ships xT/wqkv/wout as bf16; DMA lands directly in the matmul operand
    tiles (no fp32 staging or rounding copies). Whole-tensor DMAs via
    "(ko p) c -> p ko c" rearranges: one descriptor batch per issue.
  - A chain of zero-data warmup matmuls keeps the PE p-state ramping while
    the input DMA lands, so real matmuls run at full clock from the start.
  - Stage A (qkv projection) in bf16; outputs copied PSUM->SBUF as f32r:
    QT/KT in [d, seq] head-pair tiles, V_aug [seq, v|ones] per j-tile,
    q pre-scaled by dim^-0.5/temperature on host.
  - Main loop: 8 phases = (i-chunk 0..3) x (head pair 0..1), 16 j-tiles each.
    Per (phase, jt): 2 score matmuls -> one [128,1024] exp on ACT -> 2 PV
    matmuls accumulating [v|ones] into the phase's pvl bank-set. PSUM: 2
    alternating score tiles (2 banks each) + 2 pvl bank-sets (2 banks each)
    = 8 banks; the set idle in a phase is borrowed by stage-A accumulators,
    outproj tiles and the void pipeline. Stage A / outproj work is drip-fed
    through per-jt hooks sized to the ACT slack per iteration.
  - Void token (j-tile 17) is handled as a rank-1 update instead of a full
    tile: tiny [128,16] S-layout score matmuls (fp32r needs even N, so each
    score is computed twice), a [128,16] exp, PE transpose, and a DRAM-bounce
    repack to [1,512] rows on partitions 0/64 (the only legal matmul operand
    bases); [v_h|ones] x exp(s_void) then closes each pvl accumulation.
    Each phase's void pipeline runs inside the previous phase.
  - Normalize = reciprocal + multiply on DVE (single PSUM operand per
    instruction; GPSIMD cannot touch PSUM). The last phase's norm is
    column-chunked and interleaved with the tail out-projection.
  - Out projection in bf16 (osb bf16 x wout bf16): per-oc PSUM tiles on
    borrowed banks, DVE/ACT copies, half-width bf16 y DMAs. The last
    i-chunk's pair0 matmuls are pre-started inside the final phase.
  - y partials are bf16; host sums partials in fp32 and adds b_out.

Timeline-sim: 191.7us (baseline 260.1us); rel err vs reference ~4.2e-3.
"""
import sys

sys.path.insert(0, "/opt/trn_rl_repo")

import numpy as np

DIM, HEADS, B, N = 1024, 16, 2, 2048
D = DIM // HEADS          # 64
HPC = HEADS // 4          # heads per core = 4
P = 128
KO = DIM // P             # 8 k-tiles

_cache = {}


def _build():
    import concourse.mybir as mybir
    import concourse.tile as tile
    from concourse import bacc

    F32 = mybir.dt.float32
    F32R = mybir.dt.float32r
    BF16 = mybir.dt.bfloat16
    Exp = mybir.ActivationFunctionType.Exp

    nc = bacc.Bacc("TRN2", target_bir_lowering=False, debug=False)
    xT = nc.dram_tensor("xT", [DIM, N], BF16, kind="ExternalInput").ap()
    wqkv = nc.dram_tensor("wqkv", [DIM, 768], BF16, kind="ExternalInput").ap()
    wout = nc.dram_tensor("wout", [256, DIM], BF16, kind="ExternalInput").ap()
    voidk = nc.dram_tensor("voidk", [2, P], F32, kind="ExternalInput").ap()
    voidvo = nc.dram_tensor("voidvo", [2, 2, P], BF16,
                            kind="ExternalInput").ap()
    ident_in = nc.dram_tensor("ident_in", [P, P], F32R,
                              kind="ExternalInput").ap()
    y = nc.dram_tensor("y", [N, DIM], BF16, kind="ExternalOutput").ap()
    vscr = nc.dram_tensor("vscr", [8, 16, P], BF16, kind="Internal").ap()

    xT_r = xT.rearrange("(ko p) s -> p ko s", p=P)
    wqkv_r = wqkv.rearrange("(ko p) c -> p ko c", p=P)
    wout_r = wout.rearrange("(k p) c -> p k c", p=P)

    with tile.TileContext(nc) as tc:
        with tc.tile_pool(name="persist", bufs=1) as pp, \
             tc.tile_pool(name="work", bufs=1) as wp, \
             tc.tile_pool(name="psum", bufs=1, space="PSUM") as ps:

            # ---- persistent SBUF ----
            xT_bf = pp.tile([P, KO, N], BF16)
            wqkv_bf = pp.tile([P, KO, 768], BF16)
            wout_bf = pp.tile([P, 2, DIM], BF16)
            qt = pp.tile([P, 2, N], F32R)
            kt = pp.tile([P, 2, 2048], F32R)
            va = pp.tile([P, 16, 512], F32R)
            ones = pp.tile([P, D], F32)
            vkt = pp.tile([P, 2], F32)
            vktr = pp.tile([P, 2, 2], F32R)   # void key, column doubled
            ident = pp.tile([P, P], F32R)
            vones = pp.tile([P, 2, P], BF16)   # [v_h | ones] rank-1 lhsT

            # ---- DMA issues (all SP queue; priority order) ----
            nc.sync.dma_start(wqkv_bf[:, :, 0:256], wqkv_r[:, :, 0:256])
            nc.sync.dma_start(xT_bf[:, :, 0:256], xT_r[:, :, 0:256])
            nc.sync.dma_start(wqkv_bf[:, :, 256:512], wqkv_r[:, :, 256:512])
            nc.sync.dma_start(xT_bf[:, :, 256:512], xT_r[:, :, 256:512])
            nc.sync.dma_start(wqkv_bf[:, :, 512:768], wqkv_r[:, :, 512:768])
            nc.sync.dma_start(xT_bf[:, :, 512:1024], xT_r[:, :, 512:1024])
            nc.sync.dma_start(xT_bf[:, :, 1024:1536], xT_r[:, :, 1024:1536])
            nc.sync.dma_start(xT_bf[:, :, 1536:2048], xT_r[:, :, 1536:2048])
            nc.sync.dma_start(wout_bf[:], wout_r)
            nc.sync.dma_start(vkt[:], voidk.rearrange("a p -> p a"))
            nc.sync.dma_start(vones[0:1, :, :], voidvo[0:1, :, :])
            nc.sync.dma_start(vones[64:65, :, :], voidvo[1:2, :, :])
            nc.sync.dma_start(ident[:], ident_in)

            # ---- setup on Pool (keeps DVE free for stage-A copies) ----
            nc.gpsimd.memset(ones[:], 1.0)
            nc.gpsimd.tensor_copy(
                vktr[:], vkt[:, :, None].to_broadcast([P, 2, 2]))
            for jt in range(16):
                nc.gpsimd.tensor_copy(
                    va[:, jt, :].rearrange("p (h c) -> p h c", c=P)[:, :, D:P],
                    ones[:, None, :].to_broadcast([P, 4, D]))

            # ---- PE pipeline warmup: zero-data matmuls keep the tensor
            # engine busy (and its p-state ramping) while input DMA lands ----
            wsrc = pp.tile([P, 512], BF16)
            nc.vector.memset(wsrc[:].bitcast(mybir.dt.uint16), 0)

            # ---- stage A (borrows the idle pvl bank-set) ----
            st8 = {"other": 1, "slot": 0}

            def btag():
                t = f"pvl{st8['other']}{st8['slot']}"
                st8["slot"] ^= 1
                return t

            aqk_accs = {}

            def emit_aqk_part(sc, ft, part, nparts=4):
                kpp = KO // nparts
                if part == 0:
                    aqk_accs[(sc, ft)] = ps.tile([P, 512], F32, tag=btag(),
                                                 name=f"aqk_{sc}_{ft}")
                acc = aqk_accs[(sc, ft)]
                for ko in range(part * kpp, (part + 1) * kpp):
                    nc.tensor.matmul(
                        acc[:],
                        wqkv_bf[:, ko, ft * P:(ft + 1) * P],
                        xT_bf[:, ko, sc * 512:(sc + 1) * 512],
                        start=(ko == 0), stop=(ko == KO - 1),
                    )
                if part == nparts - 1:
                    if ft < 2:
                        nc.vector.tensor_copy(
                            qt[:, ft, sc * 512:(sc + 1) * 512], acc[:])
                    else:
                        nc.vector.tensor_copy(
                            kt[:, ft - 2, sc * 512:(sc + 1) * 512], acc[:])

            def emit_aqk(sc, ft):
                for part in range(4):
                    emit_aqk_part(sc, ft, part)

            def emit_aqk_halves(sc, ft):
                # startup variant: accumulate each 256-col half separately so
                # compute starts as soon as the first half-chunk of xT lands
                acc = ps.tile([P, 512], F32, tag=btag(), name=f"aqkh_{sc}_{ft}")
                for half in range(2):
                    cs = slice(sc * 512 + half * 256, sc * 512 + half * 256 + 256)
                    for ko in range(KO):
                        nc.tensor.matmul(
                            acc[:, half * 256:(half + 1) * 256],
                            wqkv_bf[:, ko, ft * P:(ft + 1) * P],
                            xT_bf[:, ko, cs],
                            start=(ko == 0), stop=(ko == KO - 1),
                        )
                if ft < 2:
                    nc.vector.tensor_copy(qt[:, ft, sc * 512:(sc + 1) * 512],
                                          acc[:])
                else:
                    nc.vector.tensor_copy(kt[:, ft - 2, sc * 512:(sc + 1) * 512],
                                          acc[:])

            def emit_av(st):
                acc = ps.tile([P, 512], F32, tag=btag(), name=f"av_{st}")
                for ko in range(KO):
                    nc.tensor.matmul(
                        acc[:, 0:256],
                        xT_bf[:, ko, st * P:(st + 1) * P],
                        wqkv_bf[:, ko, 512:768],
                        start=(ko == 0), stop=(ko == KO - 1),
                    )
                nc.vector.tensor_copy(
                    va[:, st, :].rearrange("p (h c) -> p h c", c=P)[:, :, 0:D],
                    acc[:, 0:256].rearrange("p (h c) -> p h c", c=D))

            # ---- main loop pieces ----
            sidx = {"i": 0}

            def emit_scores(ic, pair, jt):
                isl = slice(ic * 512, (ic + 1) * 512)
                jsl = slice(jt * P, (jt + 1) * P)
                i = sidx["i"]
                sidx["i"] += 1
                s = ps.tile([P, 1024], F32, tag=f"s{i % 2}",
                            name=f"s_{ic}_{pair}_{jt}")
                for hh in range(2):
                    nc.tensor.matmul(
                        s[:, hh * 512:(hh + 1) * 512],
                        kt[hh * D:(hh + 1) * D, pair, jsl],
                        qt[hh * D:(hh + 1) * D, pair, isl],
                        start=True, stop=True)
                return s

            def emit_exp_pvl(ic, pair, jt, s_cur, pvl, nxt, hook,
                             pvstop=False):
                p = wp.tile([P, 1024], F32R, tag="pexp", bufs=5,
                            name=f"p_{ic}_{pair}_{jt}")
                nc.scalar.activation(p[:], s_cur[:], Exp)
                s_nxt = emit_scores(*nxt) if nxt is not None else None
                if hook is not None:
                    hook()
                for hh in range(2):
                    h = 2 * pair + hh
                    nc.tensor.matmul(
                        pvl[hh][:],
                        va[:, jt, h * P:(h + 1) * P],
                        p[:, hh * 512:(hh + 1) * 512],
                        start=(jt == 0), stop=(jt == 15 and pvstop),
                    )
                return s_nxt

            # void key: S-layout scores [128 i-rows, 8 = (head, i-subtile)],
            # a tiny [128,8] exp, PE transpose to [8,128], then rank-1
            # [v|ones] x exp(s_void) closes the pvl accumulation
            def emit_void_scores(ic, pair):
                # fp32r matmuls need an even moving dim: compute each void
                # score twice (doubled key column), use the even columns
                vs = ps.tile([P, 512], F32, tag=btag(), name=f"vs_{ic}_{pair}")
                for hh in range(2):
                    for sub in range(4):
                        c = 2 * (hh * 4 + sub)
                        nc.tensor.matmul(
                            vs[:, c:c + 2],
                            qt[hh * D:(hh + 1) * D, pair,
                               ic * 512 + sub * P: ic * 512 + (sub + 1) * P],
                            vktr[hh * D:(hh + 1) * D, pair, :],
                            start=True, stop=True)
                return vs

            def emit_void_exp(vs, ic, pair):
                vse8 = wp.tile([P, 16], F32R, tag="vse8", bufs=2,
                               name=f"vse8_{ic}_{pair}")
                nc.scalar.activation(vse8[:], vs[:, 0:16], Exp)
                vst = ps.tile([P, 512], F32R, tag=btag(),
                              name=f"vst_{ic}_{pair}")
                nc.tensor.transpose(vst[0:16, 0:P], vse8[:], ident[:])
                vstb = wp.tile([16, P], BF16, tag="vstb", bufs=2,
                               name=f"vstb_{ic}_{pair}")
                nc.vector.tensor_copy(vstb[:], vst[0:16, 0:P])
                # repack the 8 rows into [1,512] rows on partitions 0 / 64
                # (the only legal matmul operand bases), bouncing through a
                # DRAM scratch slot (per phase, so no cross-phase hazard)
                pi = 2 * ic + pair
                nc.sync.dma_start(vscr[pi], vstb[:])
                vsty = wp.tile([P, 512], BF16, tag="vsty", bufs=3,
                               name=f"vsty_{ic}_{pair}")
                vscr_r = vscr[pi].rearrange(
                    "(x s t) c -> x t s c", x=2, t=2)[:, 0, :, :]
                for hh in range(2):
                    nc.sync.dma_start(
                        vsty[hh * D:hh * D + 1, :].rearrange(
                            "p (s c) -> p s c", c=P),
                        vscr_r[hh:hh + 1, :, :])
                return vsty

            def emit_void_pvl(pair, pvl, vsty, stop):
                for hh in range(2):
                    nc.tensor.matmul(
                        pvl[hh][:],
                        vones[hh * D:hh * D + 1, pair, :],
                        vsty[hh * D:hh * D + 1, :],
                        start=False, stop=stop)

            def emit_norm_chunk(ic, pair, pvl, osb, chunk):
                cs = slice(chunk * P, (chunk + 1) * P)
                for hh in range(2):
                    r_sb = wp.tile([D, P], F32, tag=f"rsbs{hh}", bufs=2,
                                   name=f"rsbs_{ic}_{pair}_{hh}_{chunk}")
                    nc.vector.reciprocal(r_sb[:], pvl[hh][D:P, cs])
                    nc.vector.tensor_tensor(
                        osb[hh * D:(hh + 1) * D, cs],
                        pvl[hh][0:D, cs], r_sb[:],
                        mybir.AluOpType.mult)

            def emit_norm(ic, pair, pvl):
                # BIR allows only one PSUM operand per instruction: move one
                # side to SBUF first. head0 via DVE recip+mult, head1 via
                # Pool copy+divide.
                osb = wp.tile([P, 512], BF16, tag=f"osb{pair}", bufs=2,
                              name=f"osb_{ic}_{pair}")
                for hh in range(2):
                    r_sb = wp.tile([D, 512], F32, tag=f"rsb{hh}", bufs=2,
                                   name=f"rsb_{ic}_{pair}_{hh}")
                    nc.vector.reciprocal(r_sb[:], pvl[hh][D:P, :])
                    nc.vector.tensor_tensor(osb[hh * D:(hh + 1) * D, :],
                                            pvl[hh][0:D, :], r_sb[:],
                                            mybir.AluOpType.mult)
                return osb

            def emit_outproj_oc(ic, it, oc, osbs):
                yp = ps.tile([P, 512], F32, tag=btag(),
                             name=f"y_{ic}_{it}_{oc}")
                for pair in range(2):
                    nc.tensor.matmul(
                        yp[:],
                        osbs[pair][:, it * P:(it + 1) * P],
                        wout_bf[:, pair, oc * 512:(oc + 1) * 512],
                        start=(pair == 0), stop=(pair == 1),
                    )
                ysbh = wp.tile([P, 512], BF16, tag="ysbh", bufs=6,
                               name=f"ysbh_{ic}_{it}_{oc}")
                nc.vector.tensor_copy(ysbh[:], yp[:])
                nc.sync.dma_start(
                    y[ic * 512 + it * P: ic * 512 + (it + 1) * P,
                      oc * 512:(oc + 1) * 512], ysbh[:])

            def emit_outproj_pre(ic, it, osb0):
                yps = []
                for oc in range(2):
                    yp = ps.tile([P, 512], F32, tag=btag(),
                                 name=f"y_{ic}_{it}_{oc}")
                    nc.tensor.matmul(
                        yp[:],
                        osb0[:, it * P:(it + 1) * P],
                        wout_bf[:, 0, oc * 512:(oc + 1) * 512],
                        start=True, stop=False,
                    )
                    yps.append(yp)
                return yps

            def emit_outproj_fin(ic, it, osb1, yps):
                for oc in range(2):
                    nc.tensor.matmul(
                        yps[oc][:],
                        osb1[:, it * P:(it + 1) * P],
                        wout_bf[:, 1, oc * 512:(oc + 1) * 512],
                        start=False, stop=True,
                    )
                    ysbh = wp.tile([P, 512], BF16, tag="ysbh", bufs=6,
                                   name=f"ysbh_{ic}_{it}_{oc}")
                    nc.scalar.copy(ysbh[:], yps[oc][:])
                    nc.sync.dma_start(
                        y[ic * 512 + it * P: ic * 512 + (it + 1) * P,
                          oc * 512:(oc + 1) * 512], ysbh[:])

            # ---- hook schedule ----
            osbs = {}
            yps_pre = {}

            yp_mid = {}

            def ojh(ic, k, pair):
                def f():
                    it, oc = k // 2, k % 2
                    osbs_ = [osbs[(ic, 0)], osbs[(ic, 1)]]
                    if pair == 0:
                        yp_mid[(ic, k)] = ps.tile(
                            [P, 512], F32, tag=btag(),
                            name=f"y_{ic}_{it}_{oc}")
                    yp = yp_mid[(ic, k)]
                    nc.tensor.matmul(
                        yp[:],
                        osbs_[pair][:, it * P:(it + 1) * P],
                        wout_bf[:, pair, oc * 512:(oc + 1) * 512],
                        start=(pair == 0), stop=(pair == 1),
                    )
                    if pair == 1:
                        ysbh = wp.tile([P, 512], BF16, tag="ysbh", bufs=6,
                                       name=f"ysbh_{ic}_{it}_{oc}")
                        nc.vector.tensor_copy(ysbh[:], yp[:])
                        nc.sync.dma_start(
                            y[ic * 512 + it * P: ic * 512 + (it + 1) * P,
                              oc * 512:(oc + 1) * 512], ysbh[:])
                return f

            def ojp(ic, it):
                def f():
                    yps_pre[it] = emit_outproj_pre(ic, it, osbs[(ic, 0)])
                return f

            def seq2(f1, f2):
                return lambda: (f1(), f2())

            def aqk8(sc, ft, kos):
                return lambda: [emit_aqk_part(sc, ft, k, nparts=8)
                                for k in kos]

            def aqk2(sc, ft, half):
                return lambda: [emit_aqk_part(sc, ft, 2 * half + i)
                                for i in range(2)]

            def aqk(sc, ft, part=None):
                if part is None:
                    return lambda: emit_aqk(sc, ft)
                return lambda: emit_aqk_part(sc, ft, part)

            def av(*sts):
                return lambda: [emit_av(st) for st in sts]

            hooks = {
                (0, 0): av(0, 1), (0, 1): av(2, 3), (0, 2): aqk(1, 2),
                (0, 3): av(4, 5), (0, 4): av(6, 7), (0, 5): aqk(2, 2),
                (0, 6): av(8, 9), (0, 7): av(10, 11), (0, 8): aqk(3, 2),
                (0, 9): av(12, 13), (0, 10): av(14, 15),
                (0, 11): aqk2(0, 1, 0), (0, 12): aqk2(0, 3, 0),
                (0, 13): aqk2(0, 1, 1), (0, 14): aqk2(0, 3, 1),
                (0, 15): aqk2(1, 3, 0), (1, 0): aqk2(1, 3, 1),
                (1, 2): aqk(2, 3, 0), (1, 3): aqk(2, 3, 1),
                (1, 4): aqk(2, 3, 2), (1, 5): aqk(2, 3, 3),
                (1, 6): aqk(3, 3, 0), (1, 7): aqk(3, 3, 1),
                (1, 8): aqk(3, 3, 2), (1, 9): aqk(3, 3, 3),
                (1, 10): aqk(1, 0, 0), (1, 11): aqk(1, 0, 1),
                (1, 12): aqk(1, 0, 2), (1, 13): aqk(1, 0, 3),
                (2, 2): ojh(0, 0, 0), (2, 3): ojh(0, 0, 1),
                (2, 4): ojh(0, 1, 0), (2, 5): ojh(0, 1, 1),
                (2, 6): ojh(0, 2, 0), (2, 7): ojh(0, 2, 1),
                (2, 14): ojh(0, 3, 0), (2, 15): ojh(0, 3, 1),
                (3, 2): ojh(0, 4, 0), (3, 3): ojh(0, 4, 1),
                (3, 4): ojh(0, 5, 0), (3, 5): ojh(0, 5, 1),
                (3, 6): ojh(0, 6, 0), (3, 7): ojh(0, 6, 1),
                (3, 14): ojh(0, 7, 0), (3, 15): ojh(0, 7, 1),
                (2, 8): aqk8(1, 1, [0, 1]), (2, 9): aqk8(1, 1, [2]),
                (2, 10): aqk8(1, 1, [3]), (2, 11): aqk8(1, 1, [4]),
                (2, 12): aqk8(1, 1, [5]), (2, 13): aqk8(1, 1, [6, 7]),
                (3, 8): aqk8(2, 0, [0, 1]), (3, 9): aqk8(2, 0, [2]),
                (3, 10): aqk8(2, 0, [3]), (3, 11): aqk8(2, 0, [4]),
                (3, 12): aqk8(2, 0, [5]), (3, 13): aqk8(2, 0, [6, 7]),
                (4, 2): ojh(1, 0, 0), (4, 3): ojh(1, 0, 1),
                (4, 4): ojh(1, 1, 0), (4, 5): ojh(1, 1, 1),
                (4, 6): ojh(1, 2, 0), (4, 7): ojh(1, 2, 1),
                (4, 14): ojh(1, 3, 0), (4, 15): ojh(1, 3, 1),
                (5, 2): ojh(1, 4, 0), (5, 3): ojh(1, 4, 1),
                (5, 4): ojh(1, 5, 0), (5, 5): ojh(1, 5, 1),
                (5, 6): ojh(1, 6, 0), (5, 7): ojh(1, 6, 1),
                (5, 14): ojh(1, 7, 0), (5, 15): ojh(1, 7, 1),
                (4, 8): aqk8(2, 1, [0, 1]), (4, 9): aqk8(2, 1, [2]),
                (4, 10): aqk8(2, 1, [3]), (4, 11): aqk8(2, 1, [4]),
                (4, 12): aqk8(2, 1, [5]), (4, 13): aqk8(2, 1, [6, 7]),
                (5, 8): aqk8(3, 0, [0, 1]), (5, 9): aqk8(3, 0, [2]),
                (5, 10): aqk8(3, 0, [3]), (5, 11): aqk8(3, 0, [4]),
                (5, 12): aqk8(3, 0, [5]), (5, 13): aqk8(3, 0, [6, 7]),
                (6, 2): ojh(2, 0, 0), (6, 3): ojh(2, 0, 1),
                (6, 4): ojh(2, 1, 0), (6, 5): ojh(2, 1, 1),
                (6, 6): ojh(2, 2, 0), (6, 7): ojh(2, 2, 1),
                (6, 8): ojh(2, 3, 0), (6, 9): ojh(2, 3, 1),
                (7, 0): ojh(2, 4, 0), (7, 1): ojh(2, 4, 1),
                (7, 2): ojh(2, 5, 0), (7, 3): ojh(2, 5, 1),
                (7, 4): ojh(2, 6, 0), (7, 5): ojh(2, 6, 1),
                (7, 6): ojh(2, 7, 0), (7, 7): ojh(2, 7, 1),
                (6, 10): aqk(3, 1, 0), (6, 11): aqk(3, 1, 1),
                (6, 12): aqk(3, 1, 2), (6, 13): aqk(3, 1, 3),
                (7, 9): ojp(3, 0), (7, 11): ojp(3, 1),
                (7, 13): ojp(3, 2), (7, 15): ojp(3, 3),
            }

            # ---- main schedule ----
            phases = [(ic, pair) for ic in range(4) for pair in range(2)]

            NWARM = 14
            for w in range(NWARM):
                wacc = ps.tile([P, 512], F32, tag=btag(), name=f"warm_{w}")
                nc.tensor.matmul(wacc[:], wsrc[:, 0:128], wsrc[:],
                                 start=True, stop=True)

            emit_aqk_halves(0, 0)
            emit_aqk_halves(0, 2)
            s_cur = emit_scores(0, 0, 0)
            vse_cur = vse_nxt = None
            for pi, (ic, pair) in enumerate(phases):
                sset = pi % 2
                st8["other"] = 1 - sset
                pvl = [ps.tile([P, 512], F32, tag=f"pvl{sset}{hh}",
                               name=f"pvl_{pi}_{hh}") for hh in range(2)]
                vs_cur = None
                for jt in range(16):
                    if jt == 15:
                        nxt = ((phases[pi + 1][0], phases[pi + 1][1], 0)
                               if pi < 7 else None)
                    else:
                        nxt = (ic, pair, jt + 1)
                    if jt == 15:
                        # rank-1 first so PV(15) carries the stop and
                        # normalization starts immediately
                        emit_void_pvl(pair, pvl, vse_cur, stop=False)
                    s_cur = emit_exp_pvl(ic, pair, jt, s_cur, pvl, nxt,
                                         hooks.get((pi, jt)),
                                         pvstop=True)
                    if pi == 0:
                        if jt == 1:
                            vs_cur = emit_void_scores(ic, pair)
                        elif jt == 3:
                            vse_cur = emit_void_exp(vs_cur, ic, pair)
                    if jt == 0 and pi in (2, 4, 6):
                        # PE-bound phase: void exp after jt0 so PV(0) isn't
                        # delayed behind it at the boundary
                        vse_cur = emit_void_exp(vs_nxt_pend, ic, pair)
                    if jt == 13 and pi < 7:
                        vs_nxt = emit_void_scores(*phases[pi + 1])
                vs_nxt_pend = None
                if pi < 7:
                    if pi + 1 in (2, 4, 6):
                        vs_nxt_pend = vs_nxt      # exp deferred into pi+1
                        vse_nxt = None
                    else:
                        # ACT-bound next phase: void exp fills the boundary
                        # ACT bubble
                        vse_nxt = emit_void_exp(vs_nxt, *phases[pi + 1])
                if pi == 7:
                    last_pvl = pvl
                else:
                    osbs[(ic, pair)] = emit_norm(ic, pair, pvl)
                vse_cur = vse_nxt
            # tail: interleave last-phase norm chunks with the outproj fins
            osb1 = wp.tile([P, 512], BF16, tag="osb1", bufs=2, name="osb_3_1")
            for it in range(4):
                emit_norm_chunk(3, 1, last_pvl, osb1, it)
                emit_outproj_fin(3, it, osb1, yps_pre[it])

    nc.compile()
    return nc


def _voidvo(vv4):
    """[v_h | ones] rank-1 lhsT rows for the void value: [hh, pair, 128]."""
    import ml_dtypes
    out = np.ones((2, 2, P), np.float32)
    for pair in range(2):
        for hh in range(2):
            out[hh, pair, 0:D] = vv4[2 * pair + hh]
    return out.astype(ml_dtypes.bfloat16)


def _prep_inputs(x, w_qkv, w_out, b_out, void_q, void_k, void_v,
                 attention_trace, temperature_factor):
    """Host-side sharding / layout prep. Returns in_maps for 8 cores."""
    import ml_dtypes
    BF = ml_dtypes.bfloat16

    temp = np.maximum(1.0 + np.abs(attention_trace) * temperature_factor,
                      1.0).reshape(HEADS).astype(np.float32)
    scale = (DIM ** -0.5) / temp                       # [16] per head
    qcol_scale = np.repeat(scale, D)                   # [1024]
    wq_scaled = (w_qkv[:, 0:DIM] * qcol_scale[None, :]).astype(np.float32)
    wk = w_qkv[:, DIM:2 * DIM]
    wv_full = w_qkv[:, 2 * DIM:3 * DIM]
    vk = void_k.reshape(HEADS, D)
    vv = void_v.reshape(HEADS, D)

    in_maps = []
    for core in range(8):
        b, hg = divmod(core, 4)
        h0 = hg * HPC
        cs = slice(h0 * D, (h0 + HPC) * D)             # 256 feature cols
        in_maps.append({
            "xT": np.ascontiguousarray(x[b].T).astype(BF),
            "wqkv": np.ascontiguousarray(
                np.concatenate([wq_scaled[:, cs], wk[:, cs],
                                wv_full[:, cs]], axis=1)).astype(BF),
            "wout": np.ascontiguousarray(w_out[cs, :]).astype(BF),
            "voidk": np.ascontiguousarray(vk[h0:h0 + HPC].reshape(2, P)),
            "voidvo": _voidvo(vv[h0:h0 + HPC]),
            "ident_in": np.eye(P, dtype=np.float32),
        })
    return in_maps


def _run(in_maps, trace=False):
    from concourse import bass_utils
    if "nc" not in _cache:
        _cache["nc"] = _build()
    return bass_utils.run_bass_kernel_spmd(
        _cache["nc"], in_maps, core_ids=list(range(8)), trace=trace)


def kernel(x, w_qkv, w_out, b_out, void_q, void_k, void_v,
           attention_trace, temperature_factor):
    args = [np.asarray(a, dtype=np.float32) for a in
            (x, w_qkv, w_out, b_out, void_q, void_k, void_v,
             attention_trace, temperature_factor)]
    in_maps = _prep_inputs(*args)
    res = _run(in_maps)
    out = np.zeros((B, N, DIM), np.float32)
    for core in range(8):
        b = core // 4
        out[b] += np.asarray(res.results[core]["y"], dtype=np.float32)
    out += args[3][None, None, :]                      # b_out
    return out
